# revision 33
# baseline (speedup 1.0000x reference)
"""Additive-attention kernel for TRN2, data-parallel over batch across 8 NeuronCores.

Reference computation (per batch b):
    energy[t,h] = tanh( enc[t,:] @ We[h,:] + hidden[b,:] @ Wh[h,:] + b_attn[h] )
    scores[t]   = energy[t,:] @ v
    out[b,0,:]  = softmax(scores)

Shapes: B=32, T=2048, D=1024, H=512.  W_attn = [Wh | We] : [H, 2D].

Per-core (4 batches): the dominant work is enc @ We^T (8.6 GFLOP, 33.5 MB f32
HBM traffic) -> ridge regime at bf16 TensorE throughput.

Strategy per core:
  - gpsimd (SWDGE) DMA loads enc f32 -> bf16 SBUF (cast in DMA), natural [t,d].
  - XBAR dma_start_transpose (SBUF->SBUF, bf16) produces encT [d on partitions].
  - TensorE: psum[h=128, t=512] += WeT[dchunk] (stationary) @ encT[dchunk] x8.
  - ScalarE: energy = tanh(psum + c[b,h]) with per-partition bias, writes bf16.
  - TensorE: scores psum[4, t=512] = sum_hc v[hc] @ energy[hc]  (contract over h).
  - Softmax over T on DVE/ACT, f32 out.
"""

import numpy as np
import ml_dtypes

import concourse.bass as bass
import concourse.mybir as mybir
import concourse.tile as tile
from concourse import bacc
from concourse.bass_utils import run_bass_kernel_spmd

B, T, D, H = 32, 2048, 1024, 512
NCORES = 8
BC = B // NCORES          # batches per core
TT = 512                  # t-tile (psum free dim)
NTT = T // TT             # 4 t-tiles per batch
DC = D // 128             # 8 contraction chunks
HT = H // 128             # 4 h tiles

F32 = mybir.dt.float32
BF16 = mybir.dt.bfloat16

# Of the 4 q-blocks (128 t-rows each) per t-tile, this many are transposed by
# the DMA XBAR; the rest go through TensorE transpose + PSUM->SBUF copy.
# XBAR completions kept landing on the critical path (queue FIFO + cross-engine
# coupling), so all transposes go through TensorE (56ns each, LDW-overlapped).
XQ = 0

_BUILD_CACHE = {}


def _build_nc():
    """Build the SPMD Bass graph (same on all 8 cores)."""
    nc = bacc.Bacc("TRN2", target_bir_lowering=False, debug=False,
                   num_devices=NCORES)

    enc = nc.dram_tensor("enc", [BC, T, D], F32, kind="ExternalInput").ap()
    hid = nc.dram_tensor("hid", [16, D], F32, kind="ExternalInput").ap()
    wet = nc.dram_tensor("wet", [128, DC, H], BF16, kind="ExternalInput").ap()
    wht = nc.dram_tensor("wht", [128, DC, H], BF16, kind="ExternalInput").ap()
    v4 = nc.dram_tensor("v4", [128, HT, 128], BF16, kind="ExternalInput").ap()
    bvec = nc.dram_tensor("bvec", [128, HT], F32, kind="ExternalInput").ap()
    out = nc.dram_tensor("out", [BC, T], F32, kind="ExternalOutput").ap()

    Tanh = mybir.ActivationFunctionType.Tanh
    Exp = mybir.ActivationFunctionType.Exp
    Copy = mybir.ActivationFunctionType.Copy

    with tile.TileContext(nc) as tc:
        with (
            tc.tile_pool(name="singles", bufs=1) as singles,
            tc.tile_pool(name="natf", bufs=6) as natf_pool,
            tc.tile_pool(name="nat", bufs=3) as nat_pool,
            tc.tile_pool(name="encT", bufs=3) as encT_pool,
            tc.tile_pool(name="energy", bufs=2) as en_pool,
            tc.tile_pool(name="psh", bufs=3, space="PSUM") as psh_pool,
            tc.tile_pool(name="pss", bufs=2, space="PSUM") as pss_pool,
            tc.tile_pool(name="ptr", bufs=2, space="PSUM") as ptr_pool,
            tc.tile_pool(name="psc", bufs=1, space="PSUM") as psc_pool,
            tc.tile_pool(name="small", bufs=4) as small,
        ):
            NIT = BC * NTT
            # identity for TensorE transposes (gpsimd, otherwise idle)
            ident = singles.tile([128, 128], BF16)
            from concourse.masks import make_identity
            make_identity(nc, ident)

            # scores for batch b live on partition 32*b (engine ops need
            # 32-aligned start partitions)
            scores_sb = singles.tile([128, T], F32)
            nc.vector.memset(scores_sb, 0.0)
            mparts = singles.tile([128, BC * NTT], F32)

            # --- main loop, software-pipelined emission ---
            # DVE does the upstream f32->bf16 casts + transpose drains so it
            # can run ahead; ACT handles downstream work (tanh, scores copy).
            natf_t = {}
            nat_t = {}

            encT_t = {}

            def emit_load(k):
                # per-q loads: finer DMA-queue granularity (XBAR descriptors
                # interleave sooner) and casts can start on the first 512KB
                b, tt = divmod(k, NTT)
                natf = natf_pool.tile([128, 4, D], F32)
                src = enc[b, tt * TT:(tt + 1) * TT, :].rearrange(
                    "(q p) d -> p q d", p=128)
                for q in range(4):
                    nc.sync.dma_start(out=natf[:, q, :], in_=src[:, q, :])
                natf_t[k] = natf

            def emit_cast(k):
                natf = natf_t.pop(k)
                nat = nat_pool.tile([128, 4, D], BF16)
                for q in range(4):
                    nc.vector.tensor_copy(out=nat[:, q, :], in_=natf[:, q, :])
                nat_t[k] = nat

            def emit_trans(k):
                # TensorE transposes + DVE psum->sbuf copies produce encT(k);
                # emitted ahead of iteration k-1's matmuls so the copies drain
                # while the previous iteration computes.
                nat = nat_t.pop(k)
                encT = encT_pool.tile([128, DC, TT], BF16)
                for q in range(XQ):
                    nc.sync.dma_start_transpose(
                        encT[:, :, q * 128:(q + 1) * 128],
                        nat[:, q, :],
                    )
                for q in range(XQ, 4):
                    for dg in range(2):
                        pst = ptr_pool.tile([128, 4, 128], BF16)
                        for j in range(4):
                            dc = dg * 4 + j
                            nc.tensor.transpose(
                                pst[:, j, :],
                                nat[:, q, dc * 128:(dc + 1) * 128],
                                ident,
                            )
                        nc.vector.tensor_copy(
                            out=encT[:, dg * 4:(dg + 1) * 4,
                                     q * 128:(q + 1) * 128],
                            in_=pst,
                        )
                encT_t[k] = encT

            def emit_compute(k):
                b, tt = divmod(k, NTT)
                encT = encT_t.pop(k)
                # energy = tanh(enc @ WeT + c[b]) ; psum [h=128, t=512]
                energy = en_pool.tile([128, HT, TT], BF16)
                for ht in range(HT):
                    psh = psh_pool.tile([128, TT], F32)
                    for dc in range(DC):
                        nc.tensor.matmul(
                            psh,
                            lhsT=wet_sb[:, dc, ht * 128:(ht + 1) * 128],
                            rhs=encT[:, dc, :],
                            start=(dc == 0),
                            stop=(dc == DC - 1),
                        )
                    nc.scalar.activation(
                        out=energy[:, ht, :],
                        in_=psh,
                        func=Tanh,
                        bias=c_sb[:, ht, b:b + 1],
                        scale=1.0,
                    )
                # scores[t] = energy[t,:] @ v  (contract h on partitions).
                # v is replicated across all 128 stationary columns, so every
                # psum partition carries the same scores row; read back from
                # the 32-aligned partition 32*b.
                pss = pss_pool.tile([128, TT], F32)
                for hc in range(HT):
                    nc.tensor.matmul(
                        pss,
                        lhsT=v4_sb[:, hc, :],
                        rhs=energy[:, hc, :],
                        start=(hc == 0),
                        stop=(hc == HT - 1),
                    )
                nc.scalar.activation(
                    out=scores_sb[32 * b:32 * b + 1, tt * TT:(tt + 1) * TT],
                    in_=pss[32 * b:32 * b + 1, :],
                    func=Copy,
                )
                # running per-tile max (takes the max-reduce off the tail)
                nc.vector.tensor_reduce(mparts[:, k:k + 1], pss,
                                        axis=mybir.AxisListType.X,
                                        op=mybir.AluOpType.max)

            # prologue: enc loads first so DMA starts streaming at t=0
            for k in range(4):
                emit_load(k)
            emit_cast(0)
            emit_cast(1)
            emit_trans(0)

            # replicated parameters on the ACT HWDGE queue (keeps the sync
            # queue free for the enc loads)
            wet_sb = singles.tile([128, DC, H], BF16)
            nc.scalar.dma_start(out=wet_sb, in_=wet)
            wht_sb = singles.tile([128, DC, H], BF16)
            nc.scalar.dma_start(out=wht_sb, in_=wht)
            v4_sb = singles.tile([128, HT, 128], BF16)
            nc.scalar.dma_start(out=v4_sb, in_=v4)
            b_sb = singles.tile([128, HT], F32)
            nc.scalar.dma_start(out=b_sb, in_=bvec)

            # hidden projection: c[h, b] = hidden[b,:] @ Wh[h,:] + b_attn[h]
            hid_bf = singles.tile([16, D], BF16)
            nc.gpsimd.dma_start(out=hid_bf, in_=hid)     # f32 -> bf16 cast DMA
            hidT = singles.tile([128, DC, 16], BF16)
            nc.sync.dma_start_transpose(hidT, hid_bf)    # XBAR [16,1024]->[1024,16]
            psum_c = psc_pool.tile([128, HT, BC], F32)
            for ht in range(HT):
                for dc in range(DC):
                    nc.tensor.matmul(
                        psum_c[:, ht, :],
                        lhsT=wht_sb[:, dc, ht * 128:(ht + 1) * 128],
                        rhs=hidT[:, dc, :BC],
                        start=(dc == 0),
                        stop=(dc == DC - 1),
                    )
            c_sb = singles.tile([128, HT, BC], F32)
            nc.vector.tensor_tensor(
                c_sb[:],
                psum_c[:],
                b_sb[:, :, None].to_broadcast((128, HT, BC)),
                mybir.AluOpType.add,
            )

            for k in range(NIT):
                if k + 1 < NIT:
                    emit_trans(k + 1)
                emit_compute(k)
                if k + 4 < NIT:
                    emit_load(k + 4)
                if k + 2 < NIT:
                    emit_cast(k + 2)

            # --- softmax over T (batch b on partition 32*b) ---
            # mparts[p, k] holds the max of tile k (same value on every
            # partition); batch b's max is over its k-range, written to its
            # partition 32*b.
            mx = small.tile([128, 1], F32)
            for b in range(BC):
                nc.vector.tensor_reduce(
                    mx[32 * b:32 * b + 1, :],
                    mparts[32 * b:32 * b + 1, b * NTT:(b + 1) * NTT],
                    axis=mybir.AxisListType.X, op=mybir.AluOpType.max)
            nmx = small.tile([128, 1], F32)
            nc.vector.tensor_scalar_mul(nmx, mx, -1.0)
            ex = singles.tile([128, T], F32)
            sm = small.tile([128, 1], F32)
            # accum_out gives the softmax denominator in the same ACT pass
            nc.scalar.activation(out=ex, in_=scores_sb, func=Exp, bias=nmx,
                                 scale=1.0, accum_out=sm)
            rs = small.tile([128, 1], F32)
            nc.vector.reciprocal(rs, sm)
            pr = singles.tile([128, T], F32)
            nc.vector.tensor_tensor(
                pr[:], ex[:], rs.to_broadcast((128, T)), mybir.AluOpType.mult)
            nc.sync.dma_start(
                out=out,
                in_=pr.rearrange("(g r) t -> g r t", r=32)[:, 0, :],
            )

    nc.compile()
    return nc


def _prep_shared(W_attn, b_attn, v):
    """Host-side packing of the small replicated parameters."""
    Wh = W_attn[:, :D]                      # [H, D]
    We = W_attn[:, D:]                      # [H, D]
    # wet[p, dc, h] = We[h, dc*128+p]
    wet = np.ascontiguousarray(
        We.T.reshape(DC, 128, H).transpose(1, 0, 2)).astype(ml_dtypes.bfloat16)
    wht = np.ascontiguousarray(
        Wh.T.reshape(DC, 128, H).transpose(1, 0, 2)).astype(ml_dtypes.bfloat16)
    # v4[p, hc, j] = v[hc*128+p]  (replicated over all 128 stationary columns
    # so every psum partition carries the scores row)
    v4 = np.repeat(v.reshape(HT, 128).T[:, :, None], 128, axis=2).astype(
        ml_dtypes.bfloat16)
    v4 = np.ascontiguousarray(v4)
    bvec = np.ascontiguousarray(b_attn.reshape(HT, 128).T).astype(np.float32)
    return wet, wht, v4, bvec


def _run(inputs, trace=False):
    hidden = np.asarray(inputs["hidden"], dtype=np.float32)
    enc = np.asarray(inputs["encoder_outputs"], dtype=np.float32)
    W_attn = np.asarray(inputs["W_attn"], dtype=np.float32)
    b_attn = np.asarray(inputs["b_attn"], dtype=np.float32)
    v = np.asarray(inputs["v"], dtype=np.float32)

    wet, wht, v4, bvec = _prep_shared(W_attn, b_attn, v)

    if "nc" not in _BUILD_CACHE:
        _BUILD_CACHE["nc"] = _build_nc()
    nc = _BUILD_CACHE["nc"]

    in_maps = []
    for i in range(NCORES):
        hid_pad = np.zeros((16, D), dtype=np.float32)
        hid_pad[:BC] = hidden[i * BC:(i + 1) * BC]
        in_maps.append({
            "enc": enc[i * BC:(i + 1) * BC],
            "hid": hid_pad,
            "wet": wet,
            "wht": wht,
            "v4": v4,
            "bvec": bvec,
        })

    res = run_bass_kernel_spmd(nc, in_maps, core_ids=list(range(NCORES)),
                               trace=trace)
    outs = [np.asarray(res.results[i]["out"], dtype=np.float32)
            for i in range(NCORES)]
    full = np.concatenate(outs, axis=0).reshape(B, 1, T)
    return full, res


def kernel(**inputs) -> np.ndarray:
    out, _ = _run(inputs, trace=False)
    return out


def _ensure_ntff_hook():
    """The trimmed container lacks antenv.axon_hooks; recreate it so
    run_bass_kernel_spmd(trace=True) can drive NTFF profiling via the
    libaxon_pjrt.so C ABI (same as trn_agent_boot._ntff_profile_via_ctypes).
    Only used by the dev/profiling path, never by kernel()."""
    import sys as _sys
    import types
    import ctypes
    import contextlib

    if "antenv.axon_hooks" in _sys.modules:
        return
    so_path = "/opt/axon/libaxon_pjrt.so"
    lib = ctypes.CDLL(so_path)
    if not hasattr(lib, "axon_start_nrt_profile"):
        return
    lib.axon_start_nrt_profile.argtypes = [ctypes.POINTER(ctypes.c_int64),
                                           ctypes.c_size_t]
    lib.axon_start_nrt_profile.restype = ctypes.c_int64
    lib.axon_stop_nrt_profile.argtypes = [ctypes.c_char_p]
    lib.axon_stop_nrt_profile.restype = ctypes.c_int64

    @contextlib.contextmanager
    def _hook(output_dir, device_ids):
        import jax
        jax.devices()
        if device_ids:
            ids = (ctypes.c_int64 * len(device_ids))(*device_ids)
            rc = lib.axon_start_nrt_profile(ids, len(device_ids))
        else:
            rc = lib.axon_start_nrt_profile(None, 0)
        if rc != 0:
            raise RuntimeError(f"axon_start_nrt_profile rc={rc}")
        try:
            yield
        finally:
            n = lib.axon_stop_nrt_profile(str(output_dir).encode())
            print(f"ntff profile: {n} file(s) written to {output_dir}")

    mod = types.ModuleType("antenv.axon_hooks")
    mod.get_axon_ntff_profile_hook = lambda: _hook
    mod.set_axon_ntff_profile_hook = lambda h: None
    _sys.modules["antenv.axon_hooks"] = mod


def kernel_traced(**inputs):
    """Returns (output, exec_time_ns) using the NTFF profile hook."""
    _ensure_ntff_hook()
    out, res = _run(inputs, trace=True)
    return out, res.exec_time_ns
